# revision 34
# baseline (speedup 1.0000x reference)
"""Trainium2 Bass kernel for nn_AttentionLayer (B=4, S=2048, H=16, DH=64).

Sharding: 8 cores = 4 batches x 2 head-groups (8 heads each). Each core
computes full attention for its (batch, head-group) shard; no cross-core
communication. The host pre-transposes/casts inputs, and post-normalizes
(softmax denominator division), transposes back, and adds the value bias.

v6: software-pipelined single loop, fp8 q/k projections, host-prearranged
full-bandwidth DMAs.
 - All inputs are laid out by the host in the exact [128, kt, n] SBUF
   tiling, so every load is one contiguous full-BW DMA (the strided
   rearrange loads ran at ~50GB/s and gated the ramp). qT/kT/vT are kept
   fully resident in SBUF; no staging DMAs in the steady state.
 - QKV projections are chunked generator tasks injected into the
   attention loop's slack (ScalarE's exp stream paces at ~1.15us/j-tile).
 - q/k projections run fp8e4m3 + DoubleRow (2 k-tiles per matmul);
   weights host-prescaled by 64, the 1/4096 folded into the exp scale.
 - v projection computes all 8 heads per matmul (N=512, streaming-bound).

Device dataflow per core:
  klT[d',s] = (wk8*64).T-contract kT8   (fp8 DoubleRow, PSUM f32)
  qlT[d',s] likewise; vl[j,dh'] bf16 natural
  scoresT[j,i] = sum_dh klT[dh,j]*qlT[dh,i]  (bf16, K=64, head pairs
                                              row-packed -> concurrent)
  E = exp(scoresT*0.125/4096)  (ACT, PSUM->SBUF bf16)
  E *= maskT                   (DVE multiplicative mask)
  ctxUT[dh,i] += vl_aug[j,dh].T @ E[j,i]  (ones column -> denominator row)
Output: [520, 2048] f32 = 8 heads x (64 ctxUT rows) + 8 den rows.
"""

import numpy as np
import ml_dtypes

import concourse.bass as bass
import concourse.mybir as mybir
import concourse.tile as tile
from concourse import bacc
from concourse.bass_utils import run_bass_kernel_spmd

BF16 = mybir.dt.bfloat16
FP8 = mybir.dt.float8e4
F32 = mybir.dt.float32
DR = mybir.MatmulPerfMode.DoubleRow

S = 2048      # sequence length
D = 1024      # model dim
DL = 512      # local d' (8 heads x 64)
DH = 64       # head dim
HL = 8        # local heads
KT = 8        # k-tiles over D
MT = 4        # m-tiles over DL (128 each)
SB = 4        # s blocks of 512
JT = 16       # j tiles of 128
IB = 4        # i blocks of 512

WSCALE = 64.0                 # host prescale on wq/wk (fp8 subnormals)
EXP_SCALE = 0.125 / (WSCALE * WSCALE)

_GRAPH = None


def build_graph():
    nc = bacc.Bacc("TRN2", target_bir_lowering=False, debug=False)

    # all host-prearranged to the SBUF tiling (partition dim first)
    qT8 = nc.dram_tensor("qT8", [128, IB, KT, 512], FP8, kind="ExternalInput").ap()
    kT8 = nc.dram_tensor("kT8", [128, SB, KT, 512], FP8, kind="ExternalInput").ap()
    vT = nc.dram_tensor("vT", [128, SB, KT, 512], BF16, kind="ExternalInput").ap()
    maskT = nc.dram_tensor("maskT", [S, S], BF16, kind="ExternalInput").ap()
    wq8 = nc.dram_tensor("wq8", [128, KT, DL], FP8, kind="ExternalInput").ap()
    wk8 = nc.dram_tensor("wk8", [128, KT, DL], FP8, kind="ExternalInput").ap()
    wv = nc.dram_tensor("wv", [128, KT, DL], BF16, kind="ExternalInput").ap()
    bq = nc.dram_tensor("bq", [128, MT], F32, kind="ExternalInput").ap()
    bk = nc.dram_tensor("bk", [128, MT], F32, kind="ExternalInput").ap()
    out = nc.dram_tensor("out", [DL + HL, S], BF16, kind="ExternalOutput").ap()

    with tile.TileContext(nc) as tc:
        _build_body(tc, nc, qT8, kT8, vT, maskT, wq8, wk8, wv, bq, bk, out)

    nc.compile()
    return nc


def _build_body(tc, nc, qT8, kT8, vT, maskT, wq8, wk8, wv, bq, bk, out):
    from contextlib import ExitStack

    with ExitStack() as ctx:
        const = ctx.enter_context(tc.tile_pool(name="const", bufs=1))
        acts = ctx.enter_context(tc.tile_pool(name="acts", bufs=1))
        e_pool = ctx.enter_context(tc.tile_pool(name="epool", bufs=10))
        m_pool = ctx.enter_context(tc.tile_pool(name="mpool", bufs=12))
        o_pool = ctx.enter_context(tc.tile_pool(name="opool", bufs=4))
        ppsum = ctx.enter_context(tc.tile_pool(name="ppsum", bufs=2, space="PSUM"))
        spsum = ctx.enter_context(tc.tile_pool(name="spsum", bufs=2, space="PSUM"))
        cpsum = ctx.enter_context(tc.tile_pool(name="cpsum", bufs=1, space="PSUM"))

        # ---- t=0: ACT table preload + full-BW resident loads ----
        zero_b = const.tile([128, 1], F32)
        nc.vector.memset(zero_b[:], 0.0)
        warm = const.tile([128, 1], F32)
        nc.scalar.activation(
            warm[:], zero_b[:], mybir.ActivationFunctionType.Exp,
            bias=zero_b[:], scale=1.0)
        # PE warmup burst: keep TensorE busy from t~6us so the HAM clock
        # gate opens (2.4GHz) before the real projections start
        wl = const.tile([128, 2], BF16)
        wr = const.tile([128, 512], BF16)
        nc.vector.memset(wl[:], 0.0)
        nc.vector.memset(wr[:], 0.0)
        wps = cpsum.tile([128, 512], F32, tag="c0")
        for _ in range(16):
            nc.tensor.matmul(wps[0:2, :], wl[:], wr[:], start=True, stop=True)

        wq_sb = const.tile([128, KT, DL], FP8)
        wk_sb = const.tile([128, KT, DL], FP8)
        wv_sb = const.tile([128, KT, DL], BF16)
        kT_sb = acts.tile([128, SB, KT, 512], FP8)
        qT_sb = acts.tile([128, IB, KT, 512], FP8)
        vT_sb = acts.tile([128, SB, KT, 512], BF16)
        bq_sb = const.tile([128, MT], F32)
        bk_sb = const.tile([128, MT], F32)
        # need-order on the two HWDGE queues; SWDGE (gpsimd) is ~80GB/s
        # and only carries the steady-state mask stream
        nc.sync.dma_start(out=wk_sb[:], in_=wk8)
        nc.scalar.dma_start(out=wv_sb[:], in_=wv)
        nc.sync.dma_start(out=kT_sb[:, 0], in_=kT8[:, 0])
        nc.scalar.dma_start(out=vT_sb[:, 0], in_=vT[:, 0])
        nc.sync.dma_start(out=bk_sb[:], in_=bk)
        nc.sync.dma_start(out=kT_sb[:, 1], in_=kT8[:, 1])
        nc.sync.dma_start(out=kT_sb[:, 2], in_=kT8[:, 2])
        nc.sync.dma_start(out=kT_sb[:, 3], in_=kT8[:, 3])
        nc.sync.dma_start(out=bq_sb[:], in_=bq)
        nc.sync.dma_start(out=wq_sb[:], in_=wq8)
        nc.sync.dma_start(out=qT_sb[:, 0], in_=qT8[:, 0])
        for sb in range(1, SB):
            nc.scalar.dma_start(out=vT_sb[:, sb], in_=vT[:, sb])
        for ib in range(1, IB):
            nc.sync.dma_start(out=qT_sb[:, ib], in_=qT8[:, ib])

        qlT_sb = acts.tile([128, MT, S], BF16)   # [d' partition, m-tile, s]
        klT_sb = acts.tile([128, MT, S], BF16)
        vl_sb = acts.tile([128, JT, HL, DH + 1], BF16)  # per j-tile, per head, +ones
        nc.vector.memset(vl_sb[:, :, :, DH], 1.0)

        # ---- chunked projection generators ----
        def gen_k_proj(m, sbs=tuple(range(SB))):
            msl = slice(m * 128, (m + 1) * 128)
            for sb in sbs:
                ssl = slice(sb * 512, (sb + 1) * 512)
                ps = ppsum.tile([128, 512], F32, tag="pp")
                for kk in (0, 2, 4, 6):
                    nc.tensor.matmul(
                        ps[:], wk_sb[:, kk:kk + 2, msl], kT_sb[:, sb, kk:kk + 2, :],
                        start=(kk == 0), stop=(kk == 6), perf_mode=DR)
                    yield
                nc.vector.tensor_scalar_add(
                    klT_sb[:, m, ssl], ps[:], bk_sb[:, m:m + 1])
                yield

        def gen_q_proj(m, ib):
            msl = slice(m * 128, (m + 1) * 128)
            isl = slice(ib * 512, (ib + 1) * 512)
            ps = ppsum.tile([128, 512], F32, tag="pp")
            for kk in (0, 2, 4, 6):
                nc.tensor.matmul(
                    ps[:], wq_sb[:, kk:kk + 2, msl], qT_sb[:, ib, kk:kk + 2, :],
                    start=(kk == 0), stop=(kk == 6), perf_mode=DR)
                yield
            nc.vector.tensor_scalar_add(
                qlT_sb[:, m, isl], ps[:], bq_sb[:, m:m + 1])
            yield

        def gen_v_proj_sb(sb):
            # all 8 heads at once: N=512 keeps the PE streaming-bound
            for jj in range(4):
                jt = sb * 4 + jj
                jsl = slice(jj * 128, (jj + 1) * 128)
                psv = ppsum.tile([128, 512], F32, tag="pp")
                for kk0 in (0, 4):
                    for kk in range(kk0, kk0 + 4):
                        nc.tensor.matmul(
                            psv[:], vT_sb[:, sb, kk, jsl], wv_sb[:, kk, :],
                            start=(kk == 0), stop=(kk == KT - 1))
                    yield
                nc.vector.tensor_copy(
                    vl_sb[:, jt, :, 0:DH],
                    psv[:].rearrange("p (h d) -> p h d", h=HL))
                yield

        def drain(gens):
            # round-robin so the two ppsum slots stay busy back-to-back
            # (sequential drain leaves PE gaps -> HAM re-throttles to 1.2GHz)
            gens = list(gens)
            while gens:
                g = gens.pop(0)
                try:
                    next(g)
                    gens.append(g)
                except StopIteration:
                    pass

        pending = []

        def inject(n, queue=None):
            queue = pending if queue is None else queue
            done = 0
            while queue and done < n:
                try:
                    next(queue[0])
                    done += 1
                except StopIteration:
                    queue.pop(0)

        def gen_warm_filler():
            # dep-free tiny matmuls: fill DMA-wait holes in the preamble so
            # the PE never idles >3.4us (HAM would re-throttle to 1.2GHz)
            for _ in range(40):
                nc.tensor.matmul(wps[0:2, 0:128], wl[:], wr[:, 0:128],
                                 start=True, stop=True)
                yield

        # ---- preamble: k(m0), v(s-block 0), q(m0, ib0) ----
        drain([gen_k_proj(0), gen_v_proj_sb(0), gen_q_proj(0, 0),
               gen_warm_filler()])
        vqueue = [gen_v_proj_sb(1), gen_v_proj_sb(2), gen_v_proj_sb(3)]
        pending = [gen_q_proj(0, ib) for ib in range(1, IB)]

        # ---- fused attention loop ----
        for hp in range(4):
            h0, h1 = 2 * hp, 2 * hp + 1
            if hp < 3:
                pending.append(gen_k_proj(hp + 1))
                pending.extend(gen_q_proj(hp + 1, ib) for ib in range(IB))
            for ib in range(IB):
                isl = slice(ib * 512, (ib + 1) * 512)
                ctx0 = cpsum.tile([DH + 1, 512], F32, tag="c0")
                ctx1 = cpsum.tile([DH + 1, 512], F32, tag="c1")
                pend_ctx = None
                for jt in range(JT):
                    jsl = slice(jt * 128, (jt + 1) * 128)
                    msk = m_pool.tile([128, 512], BF16, tag="msk")
                    nc.sync.dma_start(out=msk[:], in_=maskT[jsl, isl])
                    sc = spsum.tile([128, 1024], F32, tag="sc")
                    nc.tensor.matmul(
                        sc[:, 0:512],
                        klT_sb[0:64, hp, jsl], qlT_sb[0:64, hp, isl],
                        start=True, stop=True)
                    nc.tensor.matmul(
                        sc[:, 512:1024],
                        klT_sb[64:128, hp, jsl], qlT_sb[64:128, hp, isl],
                        start=True, stop=True)
                    if pend_ctx is not None:
                        pend_ctx()  # ctx of jt-1: scores lead in priority
                    E = e_pool.tile([128, 1024], BF16, tag="E")
                    nc.scalar.activation(
                        E[:], sc[:], mybir.ActivationFunctionType.Exp,
                        scale=EXP_SCALE)
                    ev = E[:].rearrange("p (o n) -> p o n", o=2)
                    mb = msk[:].rearrange("p (o n) -> p o n", o=1)
                    mb = mb.broadcast_to([128, 2, 512])
                    nc.vector.tensor_tensor(
                        ev, ev, mb, mybir.AluOpType.mult)

                    def make_ctx(jt_, E_):
                        def run():
                            nc.tensor.matmul(
                                ctx0[:], vl_sb[:, jt_, h0, :], E_[:, 0:512],
                                start=(jt_ == 0), stop=(jt_ == JT - 1))
                            nc.tensor.matmul(
                                ctx1[:], vl_sb[:, jt_, h1, :], E_[:, 512:1024],
                                start=(jt_ == 0), stop=(jt_ == JT - 1))
                        return run

                    pend_ctx = make_ctx(jt, E)
                    if vqueue:
                        inject(4, vqueue)  # hp0/ib0 ramp: feed vl production
                    else:
                        inject(1)
                pend_ctx()
                o0 = o_pool.tile([DH + 1, 512], BF16, tag="o")
                o1 = o_pool.tile([DH + 1, 512], BF16, tag="o")
                nc.vector.tensor_copy(o0[:], ctx0[:])
                nc.vector.tensor_copy(o1[:], ctx1[:])
                nc.sync.dma_start(
                    out=out[h0 * 65:h0 * 65 + 65, isl], in_=o0[:])
                nc.sync.dma_start(
                    out=out[h1 * 65:h1 * 65 + 65, isl], in_=o1[:])
        # drain any stragglers (shouldn't happen)
        while pending:
            inject(8)


def _get_graph():
    global _GRAPH
    if _GRAPH is None:
        _GRAPH = build_graph()
    return _GRAPH


def _tile128(a, inner):
    """[D, N] -> [128, D//128, N] (partition-major tiling, contiguous)."""
    d, n = a.shape
    return np.ascontiguousarray(
        a.reshape(d // 128, 128, n).transpose(1, 0, 2))


def _tile128c(a):
    """[D, S] -> [128, S//512, D//128, 512] (chunked partition-major)."""
    d, n = a.shape
    t = a.reshape(d // 128, 128, n // 512, 512)
    return np.ascontiguousarray(t.transpose(1, 2, 0, 3))


def make_in_maps(q, k, v, attention_mask, wq_kernel, wq_bias, wk_kernel,
                 wk_bias, wv_kernel, wv_bias):
    bf = ml_dtypes.bfloat16
    f8 = ml_dtypes.float8_e4m3
    in_maps = []
    for c in range(8):
        b, hg = divmod(c, 2)
        sl = slice(hg * DL, (hg + 1) * DL)
        in_maps.append({
            "qT8": _tile128c(np.asarray(q[b].T, dtype=f8)),
            "kT8": _tile128c(np.asarray(k[b].T, dtype=f8)),
            "vT": _tile128c(np.asarray(v[b].T, dtype=bf)),
            "maskT": np.asarray(attention_mask[b].T, dtype=bf),
            "wq8": _tile128(np.asarray(wq_kernel[:, sl] * WSCALE, dtype=f8), DL),
            "wk8": _tile128(np.asarray(wk_kernel[:, sl] * WSCALE, dtype=f8), DL),
            "wv": _tile128(np.asarray(wv_kernel[:, sl], dtype=bf), DL),
            "bq": np.ascontiguousarray(
                (np.asarray(wq_bias[sl], dtype=np.float32) * WSCALE)
                .reshape(MT, 128).T),
            "bk": np.ascontiguousarray(
                (np.asarray(wk_bias[sl], dtype=np.float32) * WSCALE)
                .reshape(MT, 128).T),
        })
    return in_maps


def assemble_output(results, wv_bias):
    B = 4
    out_full = np.empty((B, S, D), dtype=np.float32)
    for c in range(8):
        b, hg = divmod(c, 2)
        o = np.asarray(results[c]["out"], dtype=np.float32).reshape(
            HL, DH + 1, S)                             # [8, 65, 2048]
        ctxUT = o[:, 0:DH]
        den = o[:, DH]                             # [8, 2048]
        ctxn = ctxUT / den[:, None, :]
        out_full[b, :, hg * DL:(hg + 1) * DL] = (
            ctxn.transpose(2, 0, 1).reshape(S, DL))
    out_full += np.asarray(wv_bias, dtype=np.float32)[None, None, :]
    return out_full


def kernel(q, k, v, attention_mask, wq_kernel, wq_bias, wk_kernel, wk_bias,
           wv_kernel, wv_bias):
    nc = _get_graph()
    in_maps = make_in_maps(q, k, v, attention_mask, wq_kernel, wq_bias,
                           wk_kernel, wk_bias, wv_kernel, wv_bias)
    res = run_bass_kernel_spmd(nc, in_maps, core_ids=list(range(8)))
    return assemble_output(res.results, wv_bias)
